# revision 14
# baseline (speedup 1.0000x reference)
"""Chunked gated-linear-attention (GLA) kernel for Trainium2, 8 NeuronCores.

Math (per (b,h), per-head scalar decay lam):
    S_t = lam * S_{t-1} + k_t^T v_t ;  o_t = (q_t * SCALE) @ S_t

Chunked form with chunk size C (=128), per chunk:
    W[j,i]   = k_j . q_i                    (PE: lhsT=K^T slice, rhs=Q^T slice)
    Wm[j,i]  = W[j,i] * SCALE * lam^(i-j) * [j<=i]     (DVE mask multiply)
    O[i,:]   = sum_j Wm[j,i] V[j,:] + Qdec^T-col_i . S_prev
    S_new    = lam^C S_prev + sum_j lam^(C-1-j) k_j v_j^T

Sharding: B*H = 32 (b,h) units, 4 per core (head-parallel, no collectives).
Host prep (part of sharding): cast to fp16, pre-transpose Q/K to [D,T],
pack K|V rows so natural-layout DMA descriptors are 512B.
All matmul operands fp16 (PSUM accumulates fp32); measured rel_l2 ~6e-4.
"""

import math
from contextlib import ExitStack

import numpy as np

import concourse.bacc as bacc
import concourse.mybir as mybir
import concourse.tile as tile
from concourse.bass_utils import run_bass_kernel_spmd

B, T, H, D = 2, 2048, 16, 128
C = 128                  # chunk size along time
NCH = T // C             # 16 chunks
G = 4                    # chunks per load group
NG = NCH // G            # 4 groups
GC = G * C               # 512
NCORES = 8
U = (B * H) // NCORES    # 4 (b,h) units per core
SCALE = 0.08838834764831845
LAYER_IDX, NUM_LAYERS = 12, 32

F32 = mybir.dt.float32
F16 = mybir.dt.float16

TRACE = False            # test.py sets True to capture an NTFF profile
LAST = {}


def _slopes(n):
    def p2(m):
        start = 2.0 ** (-(2.0 ** (-(math.log2(m) - 3))))
        return [start * start**i for i in range(m)]

    if math.log2(n).is_integer():
        return p2(n)
    cp = 2 ** math.floor(math.log2(n))
    return p2(cp) + _slopes(2 * cp)[0::2][: n - cp]


def _lambdas():
    s = -np.asarray(_slopes(H), dtype=np.float64) * (
        1.0 - LAYER_IDX / (NUM_LAYERS - 1) + 1e-5
    )
    return np.exp(s)


def _build_nc():
    nc = bacc.Bacc(trn_type="TRN2", debug=False, num_devices=NCORES)

    qt = nc.dram_tensor("qt", [U, D, T], F16, kind="ExternalInput")
    kt = nc.dram_tensor("kt", [U, D, T], F16, kind="ExternalInput")
    kv = nc.dram_tensor("kv", [U, T, 2 * D], F16, kind="ExternalInput")
    s0 = nc.dram_tensor("s0", [U, D, D], F16, kind="ExternalInput")
    # maskc[j, u*C+i] = SCALE*lam_u^(i-j) for i>=j else 0
    maskc = nc.dram_tensor("maskc", [128, U * C], F16, kind="ExternalInput")
    # sdg[:, u*D:(u+1)*D] = lam_u^C * I
    sdg = nc.dram_tensor("sdg", [128, U * D], F16, kind="ExternalInput")
    # ck2[j, u] = lam_u^(C-1-j)
    ck2 = nc.dram_tensor("ck2", [128, U], F32, kind="ExternalInput")
    # qdm[d, u*GC + cc*C + i] = SCALE*lam_u^(i+1)  (same for all d, cc)
    qdm = nc.dram_tensor("qdm", [128, U * GC], F16, kind="ExternalInput")
    o = nc.dram_tensor("o", [U, T, D], F32, kind="ExternalOutput")

    with tile.TileContext(nc) as tc, ExitStack() as ctx:
        const = ctx.enter_context(tc.tile_pool(name="const", bufs=1))
        h16 = ctx.enter_context(tc.tile_pool(name="h16", bufs=2))
        outp = ctx.enter_context(tc.tile_pool(name="outp", bufs=4))
        psum = ctx.enter_context(tc.tile_pool(name="psum", bufs=2, space="PSUM"))
        state = ctx.enter_context(tc.tile_pool(name="state", bufs=2))

        mask_sb = const.tile([128, U * C], F16)
        nc.sync.dma_start(mask_sb[:], maskc[:])
        sdg_sb = const.tile([128, U * D], F16)
        nc.sync.dma_start(sdg_sb[:], sdg[:])
        ck2_sb = const.tile([128, U], F32)
        nc.sync.dma_start(ck2_sb[:], ck2[:])
        qdm_sb = const.tile([128, U * GC], F16)
        nc.sync.dma_start(qdm_sb[:], qdm[:])

        s_cur = state.tile([128, U * D], F16, tag="ssb")
        nc.sync.dma_start(
            s_cur[:].rearrange("p (u x) -> p u x", u=U),
            s0[:].rearrange("u d x -> d u x"),
        )

        for g in range(NG):
            t0 = g * GC
            qtb = h16.tile([128, U * GC], F16, tag="qtb", bufs=3)
            nc.sync.dma_start(
                qtb[:].rearrange("p (u t) -> p u t", u=U),
                qt[:, :, t0 : t0 + GC].rearrange("u d t -> d u t"),
            )
            ktb = h16.tile([128, U * GC], F16, tag="ktb", bufs=3)
            nc.sync.dma_start(
                ktb[:].rearrange("p (u t) -> p u t", u=U),
                kt[:, :, t0 : t0 + GC].rearrange("u d t -> d u t"),
            )
            kvb = h16.tile([128, U * G * 2 * D], F16, tag="kvb", bufs=3)
            for u in range(U):
                nc.sync.dma_start(
                    kvb[:, u * G * 2 * D : (u + 1) * G * 2 * D].rearrange(
                        "p (c x) -> p c x", c=G
                    ),
                    kv[u, t0 : t0 + GC, :].rearrange("(c p) x -> p c x", p=128),
                )
            kvv = kvb[:].rearrange("p (u c x d) -> p u c x d", u=U, c=G, x=2)

            qdec, kd = {}, {}
            for u in range(U):
                us = slice(u * GC, (u + 1) * GC)
                qdec_t = h16.tile([128, GC], F16, tag="qdec", bufs=8)
                nc.gpsimd.tensor_tensor(
                    qdec_t[:], qtb[:, us], qdm_sb[:, us], mybir.AluOpType.mult
                )
                kd_t = h16.tile([128, GC], F16, tag="kd", bufs=8)
                nc.vector.tensor_scalar(
                    kd_t[:].rearrange("p (c d) -> p c d", c=G),
                    kvv[:, u, :, 0, :],
                    ck2_sb[:, u : u + 1],
                    None,
                    mybir.AluOpType.mult,
                )
                qdec[u], kd[u] = qdec_t, kd_t

            for cc in range(G):
                c = g * G + cc
                w_bank = psum.tile([128, U * C], F32, tag="w")
                for u in range(U):
                    ts = slice(u * GC + cc * C, u * GC + (cc + 1) * C)
                    nc.tensor.matmul(
                        w_bank[:, u * C : (u + 1) * C],
                        lhsT=ktb[:, ts],
                        rhs=qtb[:, ts],
                        start=True,
                        stop=True,
                    )
                wm = h16.tile([128, U * C], F16, tag="wm", bufs=4)
                nc.vector.tensor_tensor(
                    wm[:], w_bank[:], mask_sb[:], mybir.AluOpType.mult
                )

                o_bank = psum.tile([128, U * D], F32, tag="o", bufs=3)
                s_bank = psum.tile([128, U * D], F32, tag="s", bufs=3)
                for u in range(U):
                    ds = slice(u * D, (u + 1) * D)
                    cs = slice(cc * C, (cc + 1) * C)
                    vsl = kvv[:, u, cc, 1, :]
                    nc.tensor.matmul(
                        o_bank[:, ds], lhsT=wm[:, u * C : (u + 1) * C],
                        rhs=vsl, start=True, stop=False,
                    )
                    nc.tensor.matmul(
                        o_bank[:, ds], lhsT=qdec[u][:, cs],
                        rhs=s_cur[:, ds], start=False, stop=True,
                    )
                    nc.tensor.matmul(
                        s_bank[:, ds], lhsT=sdg_sb[:, ds],
                        rhs=s_cur[:, ds], start=True, stop=False,
                    )
                    nc.tensor.matmul(
                        s_bank[:, ds], lhsT=kd[u][:, cs],
                        rhs=vsl, start=False, stop=True,
                    )

                s_new = state.tile([128, U * D], F16, tag="ssb")
                for u in range(U):
                    ds = slice(u * D, (u + 1) * D)
                    nc.scalar.copy(s_new[:, ds], s_bank[:, ds])
                s_cur = s_new

                ob = outp.tile([128, U * D], F32, tag="ob")
                nc.vector.tensor_copy(ob[:], o_bank[:])
                nc.scalar.dma_start(
                    o[:, c * C : (c + 1) * C, :].rearrange("u p d -> p u d"),
                    ob[:].rearrange("p (u d) -> p u d", u=U),
                )

    nc.compile()
    return nc


_NC_CACHE = []


def _get_nc():
    if not _NC_CACHE:
        _NC_CACHE.append(_build_nc())
    return _NC_CACHE[0]


def _core_consts(core):
    lam = _lambdas()
    i_idx = np.arange(C)
    maskc = np.zeros((128, U * C), np.float16)
    sdg = np.zeros((128, U * D), np.float16)
    ck2 = np.zeros((128, U), np.float32)
    qdm = np.zeros((128, U * GC), np.float16)
    eye = np.eye(128, dtype=np.float64)
    for u in range(U):
        h = (U * core + u) % H
        l = lam[h]
        m = np.where(
            i_idx[None, :] >= i_idx[:, None],
            SCALE * l ** (i_idx[None, :] - i_idx[:, None]),
            0.0,
        )
        maskc[:, u * C : (u + 1) * C] = m.astype(np.float16)
        sdg[:, u * D : (u + 1) * D] = (l**C * eye).astype(np.float16)
        ck2[:, u] = (l ** (C - 1 - i_idx)).astype(np.float32)
        cq = (SCALE * l ** (i_idx + 1)).astype(np.float16)  # [C]
        qdm[:, u * GC : (u + 1) * GC] = np.tile(cq, (128, G))
    return maskc, sdg, ck2, qdm


def kernel(query_states, key_states, value_states, initial_state):
    q16 = np.asarray(query_states).astype(np.float16)
    k16 = np.asarray(key_states).astype(np.float16)
    v16 = np.asarray(value_states).astype(np.float16)
    # [B,T,H,D] -> [B*H, T, D]
    q16 = np.transpose(q16, (0, 2, 1, 3)).reshape(B * H, T, D)
    k16 = np.transpose(k16, (0, 2, 1, 3)).reshape(B * H, T, D)
    v16 = np.transpose(v16, (0, 2, 1, 3)).reshape(B * H, T, D)
    s016 = np.asarray(initial_state).astype(np.float16).reshape(B * H, D, D)

    nc = _get_nc()
    in_maps = []
    for core in range(NCORES):
        lo = U * core
        maskc, sdg, ck2, qdm = _core_consts(core)
        in_maps.append(
            {
                "qt": np.ascontiguousarray(
                    q16[lo : lo + U].transpose(0, 2, 1)
                ),
                "kt": np.ascontiguousarray(
                    k16[lo : lo + U].transpose(0, 2, 1)
                ),
                "kv": np.ascontiguousarray(
                    np.concatenate([k16[lo : lo + U], v16[lo : lo + U]], axis=2)
                ),
                "s0": np.ascontiguousarray(s016[lo : lo + U]),
                "maskc": maskc,
                "sdg": sdg,
                "ck2": ck2,
                "qdm": qdm,
            }
        )

    res = run_bass_kernel_spmd(
        nc, in_maps, core_ids=list(range(NCORES)), trace=TRACE
    )
    if TRACE:
        LAST["exec_time_ns"] = res.exec_time_ns
        LAST["mean_exec_time_ns"] = res.mean_exec_time_ns
        LAST["trace"] = (
            res.instructions_and_trace[1] if res.instructions_and_trace else None
        )

    out = np.empty((B * H, T, D), np.float32)
    for core in range(NCORES):
        out[U * core : U * core + U] = res.results[core]["o"]
    return np.ascontiguousarray(
        np.transpose(out.reshape(B, H, T, D), (0, 2, 1, 3))
    )


# revision 15
# speedup vs baseline: 1.3132x; 1.3132x over previous
"""Chunked gated-linear-attention (GLA) kernel for Trainium2, 8 NeuronCores.

Math (per (b,h), per-head scalar decay lam):
    S_t = lam * S_{t-1} + k_t^T v_t ;  o_t = (q_t * SCALE) @ S_t

Chunked form with chunk size C (=128), per chunk:
    W[j,i]   = k_j . q_i                    (PE: lhsT=K^T slice, rhs=Q^T slice)
    Wm[j,i]  = W[j,i] * SCALE * lam^(i-j) * [j<=i]     (DVE mask multiply)
    O[i,:]   = sum_j Wm[j,i] V[j,:] + Qdec^T-col_i . S_prev
    S_new    = lam^C S_prev + sum_j lam^(C-1-j) k_j v_j^T

Sharding: B*H = 32 (b,h) units, 4 per core (head-parallel, no collectives).
Host prep (part of sharding): cast to fp16, pre-transpose Q/K to [D,T],
pack K|V rows so natural-layout DMA descriptors are 512B.
All matmul operands fp16 (PSUM accumulates fp32); measured rel_l2 ~6e-4.
"""

import math
from contextlib import ExitStack

import numpy as np

import concourse.bacc as bacc
import concourse.mybir as mybir
import concourse.tile as tile
from concourse.bass_utils import run_bass_kernel_spmd

B, T, H, D = 2, 2048, 16, 128
C = 128                  # chunk size along time
NCH = T // C             # 16 chunks
G = 4                    # chunks per load group
NG = NCH // G            # 4 groups
GC = G * C               # 512
NCORES = 8
U = (B * H) // NCORES    # 4 (b,h) units per core
SCALE = 0.08838834764831845
LAYER_IDX, NUM_LAYERS = 12, 32

F32 = mybir.dt.float32
F16 = mybir.dt.float16

TRACE = False            # test.py sets True to capture an NTFF profile
LAST = {}


def _slopes(n):
    def p2(m):
        start = 2.0 ** (-(2.0 ** (-(math.log2(m) - 3))))
        return [start * start**i for i in range(m)]

    if math.log2(n).is_integer():
        return p2(n)
    cp = 2 ** math.floor(math.log2(n))
    return p2(cp) + _slopes(2 * cp)[0::2][: n - cp]


def _lambdas():
    s = -np.asarray(_slopes(H), dtype=np.float64) * (
        1.0 - LAYER_IDX / (NUM_LAYERS - 1) + 1e-5
    )
    return np.exp(s)


def _build_nc():
    nc = bacc.Bacc(trn_type="TRN2", debug=False, num_devices=NCORES)

    qt = nc.dram_tensor("qt", [U, D, T], F16, kind="ExternalInput")
    kt = nc.dram_tensor("kt", [U, D, T], F16, kind="ExternalInput")
    kv = nc.dram_tensor("kv", [U, T, 2 * D], F16, kind="ExternalInput")
    s0 = nc.dram_tensor("s0", [U, D, D], F16, kind="ExternalInput")
    # maskc[j, u*C+i] = SCALE*lam_u^(i-j) for i>=j else 0
    maskc = nc.dram_tensor("maskc", [128, U * C], F16, kind="ExternalInput")
    # sdg[:, u*D:(u+1)*D] = lam_u^C * I
    sdg = nc.dram_tensor("sdg", [128, U * D], F16, kind="ExternalInput")
    # ck2[j, u] = lam_u^(C-1-j)
    ck2 = nc.dram_tensor("ck2", [128, U], F32, kind="ExternalInput")
    # qdm[d, u*GC + cc*C + i] = SCALE*lam_u^(i+1)  (same for all d, cc)
    qdm = nc.dram_tensor("qdm", [128, U * GC], F16, kind="ExternalInput")
    o = nc.dram_tensor("o", [U, T, D], F32, kind="ExternalOutput")

    with tile.TileContext(nc) as tc, ExitStack() as ctx:
        const = ctx.enter_context(tc.tile_pool(name="const", bufs=1))
        h16 = ctx.enter_context(tc.tile_pool(name="h16", bufs=2))
        outp = ctx.enter_context(tc.tile_pool(name="outp", bufs=4))
        psum = ctx.enter_context(tc.tile_pool(name="psum", bufs=2, space="PSUM"))
        state = ctx.enter_context(tc.tile_pool(name="state", bufs=2))

        mask_sb = const.tile([128, U * C], F16)
        nc.sync.dma_start(mask_sb[:], maskc[:])
        sdg_sb = const.tile([128, U * D], F16)
        nc.sync.dma_start(sdg_sb[:], sdg[:])
        ck2_sb = const.tile([128, U], F32)
        nc.sync.dma_start(ck2_sb[:], ck2[:])
        qdm_sb = const.tile([128, U * GC], F16)
        nc.sync.dma_start(qdm_sb[:], qdm[:])

        s_cur = state.tile([128, U * D], F16, tag="ssb")
        nc.sync.dma_start(
            s_cur[:].rearrange("p (u x) -> p u x", u=U),
            s0[:].rearrange("u d x -> d u x"),
        )

        for g in range(NG):
            t0 = g * GC
            qtb = h16.tile([128, U * GC], F16, tag="qtb", bufs=3)
            nc.sync.dma_start(
                qtb[:].rearrange("p (u t) -> p u t", u=U),
                qt[:, :, t0 : t0 + GC].rearrange("u d t -> d u t"),
            )
            ktb = h16.tile([128, U * GC], F16, tag="ktb", bufs=3)
            nc.sync.dma_start(
                ktb[:].rearrange("p (u t) -> p u t", u=U),
                kt[:, :, t0 : t0 + GC].rearrange("u d t -> d u t"),
            )
            kvb = h16.tile([128, U * G * 2 * D], F16, tag="kvb", bufs=3)
            for u in range(U):
                nc.sync.dma_start(
                    kvb[:, u * G * 2 * D : (u + 1) * G * 2 * D].rearrange(
                        "p (c x) -> p c x", c=G
                    ),
                    kv[u, t0 : t0 + GC, :].rearrange("(c p) x -> p c x", p=128),
                )
            kvv = kvb[:].rearrange("p (u c x d) -> p u c x d", u=U, c=G, x=2)

            qdec, kd = {}, {}
            for u in range(U):
                us = slice(u * GC, (u + 1) * GC)
                qdec_t = h16.tile([128, GC], F16, tag="qdec", bufs=8)
                nc.gpsimd.tensor_tensor(
                    qdec_t[:], qtb[:, us], qdm_sb[:, us], mybir.AluOpType.mult
                )
                kd_t = h16.tile([128, GC], F16, tag="kd", bufs=8)
                nc.vector.tensor_scalar(
                    kd_t[:].rearrange("p (c d) -> p c d", c=G),
                    kvv[:, u, :, 0, :],
                    ck2_sb[:, u : u + 1],
                    None,
                    mybir.AluOpType.mult,
                )
                qdec[u], kd[u] = qdec_t, kd_t

            for cc in range(G):
                c = g * G + cc
                w_bank = psum.tile([128, U * C], F32, tag="w")
                for u in range(U):
                    ts = slice(u * GC + cc * C, u * GC + (cc + 1) * C)
                    nc.tensor.matmul(
                        w_bank[:, u * C : (u + 1) * C],
                        lhsT=ktb[:, ts],
                        rhs=qtb[:, ts],
                        start=True,
                        stop=True,
                    )
                wm = h16.tile([128, U * C], F16, tag="wm", bufs=4)
                nc.vector.tensor_tensor(
                    wm[:], w_bank[:], mask_sb[:], mybir.AluOpType.mult
                )

                o_bank = psum.tile([128, U * D], F32, tag="o", bufs=3)
                s_bank = psum.tile([128, U * D], F32, tag="s", bufs=3)
                for u in range(U):
                    ds = slice(u * D, (u + 1) * D)
                    cs = slice(cc * C, (cc + 1) * C)
                    vsl = kvv[:, u, cc, 1, :]
                    nc.tensor.matmul(
                        o_bank[:, ds], lhsT=wm[:, u * C : (u + 1) * C],
                        rhs=vsl, start=True, stop=False,
                    )
                    nc.tensor.matmul(
                        o_bank[:, ds], lhsT=qdec[u][:, cs],
                        rhs=s_cur[:, ds], start=False, stop=True,
                    )
                    nc.tensor.matmul(
                        s_bank[:, ds], lhsT=sdg_sb[:, ds],
                        rhs=s_cur[:, ds], start=True, stop=False,
                    )
                    nc.tensor.matmul(
                        s_bank[:, ds], lhsT=kd[u][:, cs],
                        rhs=vsl, start=False, stop=True,
                    )

                s_new = state.tile([128, U * D], F16, tag="ssb")
                half = U * D // 2
                nc.scalar.copy(s_new[:, 0:half], s_bank[:, 0:half])
                nc.vector.tensor_copy(s_new[:, half:], s_bank[:, half:])
                s_cur = s_new

                ob = outp.tile([128, U * D], F32, tag="ob")
                nc.vector.tensor_copy(ob[:], o_bank[:])
                nc.scalar.dma_start(
                    o[:, c * C : (c + 1) * C, :].rearrange("u p d -> p u d"),
                    ob[:].rearrange("p (u d) -> p u d", u=U),
                )

    nc.compile()
    return nc


_NC_CACHE = []


def _get_nc():
    if not _NC_CACHE:
        _NC_CACHE.append(_build_nc())
    return _NC_CACHE[0]


def _core_consts(core):
    lam = _lambdas()
    i_idx = np.arange(C)
    maskc = np.zeros((128, U * C), np.float16)
    sdg = np.zeros((128, U * D), np.float16)
    ck2 = np.zeros((128, U), np.float32)
    qdm = np.zeros((128, U * GC), np.float16)
    eye = np.eye(128, dtype=np.float64)
    for u in range(U):
        h = (U * core + u) % H
        l = lam[h]
        m = np.where(
            i_idx[None, :] >= i_idx[:, None],
            SCALE * l ** (i_idx[None, :] - i_idx[:, None]),
            0.0,
        )
        maskc[:, u * C : (u + 1) * C] = m.astype(np.float16)
        sdg[:, u * D : (u + 1) * D] = (l**C * eye).astype(np.float16)
        ck2[:, u] = (l ** (C - 1 - i_idx)).astype(np.float32)
        cq = (SCALE * l ** (i_idx + 1)).astype(np.float16)  # [C]
        qdm[:, u * GC : (u + 1) * GC] = np.tile(cq, (128, G))
    return maskc, sdg, ck2, qdm


def kernel(query_states, key_states, value_states, initial_state):
    q16 = np.asarray(query_states).astype(np.float16)
    k16 = np.asarray(key_states).astype(np.float16)
    v16 = np.asarray(value_states).astype(np.float16)
    # [B,T,H,D] -> [B*H, T, D]
    q16 = np.transpose(q16, (0, 2, 1, 3)).reshape(B * H, T, D)
    k16 = np.transpose(k16, (0, 2, 1, 3)).reshape(B * H, T, D)
    v16 = np.transpose(v16, (0, 2, 1, 3)).reshape(B * H, T, D)
    s016 = np.asarray(initial_state).astype(np.float16).reshape(B * H, D, D)

    nc = _get_nc()
    in_maps = []
    for core in range(NCORES):
        lo = U * core
        maskc, sdg, ck2, qdm = _core_consts(core)
        in_maps.append(
            {
                "qt": np.ascontiguousarray(
                    q16[lo : lo + U].transpose(0, 2, 1)
                ),
                "kt": np.ascontiguousarray(
                    k16[lo : lo + U].transpose(0, 2, 1)
                ),
                "kv": np.ascontiguousarray(
                    np.concatenate([k16[lo : lo + U], v16[lo : lo + U]], axis=2)
                ),
                "s0": np.ascontiguousarray(s016[lo : lo + U]),
                "maskc": maskc,
                "sdg": sdg,
                "ck2": ck2,
                "qdm": qdm,
            }
        )

    res = run_bass_kernel_spmd(
        nc, in_maps, core_ids=list(range(NCORES)), trace=TRACE
    )
    if TRACE:
        LAST["exec_time_ns"] = res.exec_time_ns
        LAST["mean_exec_time_ns"] = res.mean_exec_time_ns
        LAST["trace"] = (
            res.instructions_and_trace[1] if res.instructions_and_trace else None
        )

    out = np.empty((B * H, T, D), np.float32)
    for core in range(NCORES):
        out[U * core : U * core + U] = res.results[core]["o"]
    return np.ascontiguousarray(
        np.transpose(out.reshape(B, H, T, D), (0, 2, 1, 3))
    )
